# revision 41
# baseline (speedup 1.0000x reference)
"""GCN layer (message passing + linear + ReLU) on 8 Trainium2 NeuronCores.

out = relu(((scatter_add(h[src] -> dst) + x) * dis) @ W.T),
h = x * dis,  dis = rsqrt(deg + 1),  deg = in-degree via dst counts.

Strategy (SPMD, one program on 8 cores):
  - Nodes sharded contiguously: core c owns rows [c*6250, (c+1)*6250).
  - Host partitions edges by dst owner and sorts by dst (index-only work);
    degree reaches the device as CSR rowptr slices, so deg = rowptr diff
    and dis = 1/sqrt(deg+1) are computed on device in f32.
  - The gather table is h = x*dis in bf16 (host-prescaled, O(N) prep like
    the reference's h): each edge's h[src] is one 256B dma_gather row, so
    the scatter matrix S is a pure one-hot (slot==iota) built with a
    single DVE is_equal per group -- no per-edge scale multiply.
  - Scatter-add on-chip: edges sorted by dst fall into windows of 128
    owned nodes; per 128-edge chunk S[e, slot] = (slot==dst-base) is
    built on DVE and the PE accumulates gt.T @ S = agg.T [feat, slot]
    into the window's PSUM tile.
  - Windows are processed in GROUPS of WG: within a (group, pass) the
    chunk runs of all WG windows are packed back-to-back so gather calls
    (<=1024 idx each) span window boundaries.  This cuts the number of
    dma_gather instructions from ~131 to ~85: every Q7 core pair walks
    the whole GpSimd instruction stream (non-owning pairs pay an
    idle-skip per instruction), so fewer+fuller calls raise the pairs'
    useful descriptor-generation throughput, which paces the kernel.
  - Edges inside each (pass, window) block are sorted by src so the
    random 256B HBM reads of a call walk ascending addresses (better
    row-buffer/bank locality on the drain side).
  - int16 gather indices can't span 50k rows, so each window's edges are
    processed against table halves h[:32768] / h[32768:] (pass lo/hi).
  - Gather calls round-robin over 4 SWDGE queues (queue q is served by Q7
    cores 2q/2q+1 -- tx/rx descriptor streams in parallel).
  - Pad gather slots use idx 0 (a real row, masked by S=0).  -1 pad +
    ucode trailing-trim was tried and WEDGES THE DEVICE: the NX decode
    reserves ring space from num_idxs_reg (untrimmed) while the Q7
    writes fewer descriptors, desyncing the SWDGE ring tail so SDMA
    executes stale descriptor slots.  Trimming needs per-core count
    registers, which stall the Pool decode pipeline (baseline note).
  - gidx/xst uploads are split so the first gather/finalize only waits
    for a small first segment.
  - Finalize per window, fused right after its last matmul: att =
    (psum + xT) in bf16, po = att.T @ W.T via PE (no transpose needed:
    agg is feature-major), out = relu(po * dis_dst) via ACT per-partition
    scale, then DMA out.
Chunk counts per (pass, window) are maxed over cores so the single SPMD
program fits every core; shorter cores pad with slot=255 / idx=0 chunks.
"""
import numpy as np
import ml_dtypes

from concourse import bacc, bass, mybir, tile
from concourse.bass_utils import run_bass_kernel_spmd

F32 = mybir.dt.float32
BF16 = mybir.dt.bfloat16
I32 = mybir.dt.int32
I16 = mybir.dt.int16
AF = mybir.ActivationFunctionType
OP = mybir.AluOpType

N = 50000
E = 600000
D = 128
C = 8                      # cores
NPC = N // C               # 6250 nodes per core
WPC = (NPC + 127) // 128   # 49 windows per core
NPAD = WPC * 128           # 6272 padded shard rows
NT_G = (N + 127) // 128    # 391 global node tiles
NROWS = NT_G * 128         # 50048 padded table rows
SPLIT = 32768              # src table split for int16 gather indices
PASS_BOUNDS = [(0, SPLIT), (SPLIT, N)]
GB = 8                     # max chunks per dma_gather call (1024 idxs; >1024
                           # descriptors per SWDGE call crashes the device)
SG = 16                    # chunks per S-group build
NQ = 4                     # SWDGE queues used round-robin
WG = 8                     # windows per processing group (gather-call packing
                           # unit; quad-packed PSUM accumulators)
XST_PARTS = 7              # xst upload split (7 windows each)


def _groups():
    return [list(range(g0, min(g0 + WG, WPC))) for g0 in range(0, WPC, WG)]


def _chunk_layout(K):
    """Chunk layout in (group, pass, window) order.

    Returns cbase[p, w] = global chunk index of window w's pass-p block,
    runs[(g, p)] = (start_chunk, n_chunks) of the packed per-group run,
    and TC = total chunks."""
    K = np.asarray(K)
    cbase = np.zeros((2, WPC), np.int64)
    runs = {}
    cb = 0
    for g, ws in enumerate(_groups()):
        for p in range(2):
            r0 = cb
            for w in ws:
                cbase[p, w] = cb
                cb = int(cb + K[p, w])
            runs[(g, p)] = (int(r0), int(cb - r0))
    return cbase, runs, int(cb)


# ---------------------------------------------------------------- host prep
def host_prep(edge_index):
    src = np.asarray(edge_index[0], dtype=np.int64)
    dst = np.asarray(edge_index[1], dtype=np.int64)
    order = np.argsort(dst, kind="stable")
    ss_all = src[order]
    dd_all = dst[order]
    counts = np.bincount(dst, minlength=N)
    rowptr = np.zeros(N + 1, np.int64)
    rowptr[1:] = np.cumsum(counts)
    dis = 1.0 / np.sqrt(counts.astype(np.float64) + 1.0)  # rsqrt(deg+1)

    per_core = []
    need = np.zeros((C, 2, WPC), np.int64)
    for c in range(C):
        e0, e1 = rowptr[c * NPC], rowptr[(c + 1) * NPC]
        ss, dd = ss_all[e0:e1], dd_all[e0:e1]
        per_core.append((ss, dd))
        for p, (lo, hi) in enumerate(PASS_BOUNDS):
            m = (ss >= lo) & (ss < hi)
            w = (dd[m] - c * NPC) // 128
            need[c, p] = np.bincount(w, minlength=WPC)
    # Each core processes ITS windows sorted by edge count (descending).
    # Aligning the order statistics across cores shrinks the max-over-core
    # chunk counts K, and the final processed window is everyone's
    # smallest (shorter tail after the last gather).
    perm = np.argsort(-need.sum(axis=1), axis=1, kind="stable")  # [C, WPC]
    need_s = np.take_along_axis(need, perm[:, None, :], axis=2)
    K = np.ceil(need_s.max(axis=0) / 128).astype(np.int64)  # [2, WPC]

    cbase, runs, TC = _chunk_layout(K)
    TC8 = ((TC + SG - 1) // SG) * SG

    cores = []
    for c in range(C):
        ss, dd = per_core[c]
        slot_of_w = np.empty(WPC, np.int64)
        slot_of_w[perm[c]] = np.arange(WPC)
        g = np.zeros(TC * 128, np.int64)
        s = np.full(TC8 * 128, 255, np.int64)
        for p, (lo, hi) in enumerate(PASS_BOUNDS):
            m = (ss >= lo) & (ss < hi)
            sg = ss[m]                       # global src id
            dloc = dd[m] - c * NPC
            wslot = slot_of_w[dloc // 128]
            # per-window blocks, src-ascending inside each block
            o2 = np.lexsort((sg, wslot))
            sg, dloc, wslot = sg[o2], dloc[o2], wslot[o2]
            cnt = np.bincount(wslot, minlength=WPC)
            ofs = np.zeros(WPC, np.int64)
            np.cumsum(cnt[:-1], out=ofs[1:])
            pos = cbase[p, wslot] * 128 + (np.arange(len(sg)) - ofs[wslot])
            g[pos] = sg - lo                 # index into table half
            s[pos] = dloc - (dloc // 128) * 128
        d = {}
        # gather idx layout [128, TC*8]: stream pos j at [j%16, j//16],
        # replicated across the 8 groups of 16 partitions (each SWDGE queue's
        # tx/rx Q7 core pair reads its own partition group).
        d["gidx"] = np.tile(g.reshape(-1, 16).T.astype(np.int16), (8, 1)).copy()
        # slot layout [128, TC8]: stream pos j at [j%128, j//128]; values
        # <= 255 are exact in bf16 so the S-build is a single is_equal.
        d["slots"] = s.reshape(-1, 128).T.astype(ml_dtypes.bfloat16).copy()
        n0 = c * NPC
        rpv = np.full(NPAD + 1, rowptr[min((c + 1) * NPC, N)], np.int64)
        rpv[: NPC + 1] = rowptr[n0 : n0 + NPC + 1]
        rp0m = rpv[:NPAD].reshape(WPC, 128)[perm[c]]
        rp1m = rpv[1 : NPAD + 1].reshape(WPC, 128)[perm[c]]
        d["rp0s"] = rp0m.T.astype(np.int32).copy()
        d["rp1s"] = rp1m.T.astype(np.int32).copy()
        d["perm"] = perm[c]
        cores.append(d)
    return dict(K=K, TC=TC, TC8=TC8, cores=cores, dis=dis)


# ---------------------------------------------------------------- program
def build_program(K):
    K = np.asarray(K)
    cbase, runs, TC = _chunk_layout(K)
    TC8 = ((TC + SG - 1) // SG) * SG
    groups = _groups()

    # owner window of every global chunk (program-fixed)
    owner = np.zeros(TC, np.int64)
    for w in range(WPC):
        for p in range(2):
            owner[cbase[p, w] : cbase[p, w] + K[p, w]] = w

    # gidx upload parts over group ranges
    ngrp = len(groups)
    splits = [s for s in [0, 1, 2, 4, ngrp] if s <= ngrp]
    if splits[-1] != ngrp:
        splits.append(ngrp)
    gparts = []
    for i in range(len(splits) - 1):
        ga, gb_ = splits[i], splits[i + 1]
        c0 = runs[(ga, 0)][0]
        c1 = TC if gb_ >= ngrp else runs[(gb_, 0)][0]
        gparts.append((ga, gb_, c0, c1))

    nc = bacc.Bacc(
        None, target_bir_lowering=False, debug=False, num_swdge_queues=NQ
    )

    x_p = nc.dram_tensor("xb", [NROWS, D], BF16, kind="ExternalInput")
    xst_p = nc.dram_tensor("xst", [D, NPAD], BF16, kind="ExternalInput")
    wt_p = nc.dram_tensor("wt", [D, D], BF16, kind="ExternalInput")
    iota8_p = nc.dram_tensor("iota8", [128, 128 * SG], BF16, kind="ExternalInput")
    rp0s_p = nc.dram_tensor("rp0s", [128, WPC], I32, kind="ExternalInput")
    rp1s_p = nc.dram_tensor("rp1s", [128, WPC], I32, kind="ExternalInput")
    gidx_p = nc.dram_tensor("gidx", [128, TC * 8], I16, kind="ExternalInput")
    slots_p = nc.dram_tensor("slots", [128, TC8], BF16, kind="ExternalInput")
    out_p = nc.dram_tensor("out", [NPAD, D], BF16, kind="ExternalOutput")

    with tile.TileContext(nc) as tc:
        with (
            tc.tile_pool(name="const", bufs=1) as cpool,
            tc.tile_pool(name="gather", bufs=20) as gpool,
            tc.tile_pool(name="sel", bufs=6) as spool,
            tc.tile_pool(name="fin", bufs=3) as fpool,
            tc.tile_pool(name="psA", bufs=4, space="PSUM") as psA,
            tc.tile_pool(name="psO", bufs=2, space="PSUM") as psO,
        ):
            # --- uploads; gidx parts on the sync queue (first part first so
            # gathers start early), metadata then xst on the scalar queue.
            # slots + iota go first on the scalar queue: they gate every
            # S-group build.
            gidx_sb = {}
            part_of_g = {}
            for i, (ga, gb_, c0, c1) in enumerate(gparts):
                gt_ = cpool.tile([128, (c1 - c0) * 8], I16, tag=f"gidx{i}")
                gidx_sb[i] = (gt_, c0)
                for gg in range(ga, gb_):
                    part_of_g[gg] = i
            nc.sync.dma_start(
                gidx_sb[0][0][:], gidx_p[:, gparts[0][2] * 8 : gparts[0][3] * 8]
            )
            sf = cpool.tile([128, TC8], BF16, tag="sf")
            nc.scalar.dma_start(sf[:], slots_p[:])
            iota8_sb = cpool.tile([128, 128 * SG], BF16, tag="iota8")
            nc.scalar.dma_start(iota8_sb[:], iota8_p[:])
            wt_sb = cpool.tile([128, 128], BF16, tag="wt")
            nc.scalar.dma_start(wt_sb[:], wt_p[:])
            r0i = cpool.tile([128, WPC], I32, tag="r0i")
            nc.scalar.dma_start(r0i[:], rp0s_p[:])
            r1i = cpool.tile([128, WPC], I32, tag="r1i")
            nc.scalar.dma_start(r1i[:], rp1s_p[:])

            # dis_dst = 1/sqrt(deg+1) from rowptr diffs, [128, WPC] f32
            r0f = cpool.tile([128, WPC], F32, tag="r0f")
            nc.vector.tensor_copy(r0f[:], r0i[:])
            r1f = cpool.tile([128, WPC], F32, tag="r1f")
            nc.vector.tensor_copy(r1f[:], r1i[:])
            dg = cpool.tile([128, WPC], F32, tag="dg")
            nc.vector.tensor_tensor(out=dg[:], in0=r1f[:], in1=r0f[:], op=OP.subtract)
            nc.vector.tensor_scalar_add(out=dg[:], in0=dg[:], scalar1=1.0)
            rc = cpool.tile([128, WPC], F32, tag="rc")
            nc.vector.reciprocal(rc[:], dg[:])
            dis_s = cpool.tile([128, WPC], F32, tag="dis")
            nc.scalar.activation(dis_s[:], rc[:], AF.Sqrt)

            # remaining gidx parts (sync queue), xst parts (scalar queue)
            for i in range(1, len(gparts)):
                t, c0 = gidx_sb[i]
                nc.sync.dma_start(t[:], gidx_p[:, c0 * 8 : gparts[i][3] * 8])
            xst_sb = cpool.tile([128, NPAD], BF16, tag="xst")
            xw = (WPC + XST_PARTS - 1) // XST_PARTS  # windows per xst part
            for i in range(XST_PARTS):
                a, b = i * xw * 128, min((i + 1) * xw * 128, NPAD)
                nc.scalar.dma_start(xst_sb[:, a:b], xst_p[:, a:b])

            tables = [x_p[0:SPLIT, :], x_p[SPLIT:NROWS, :]]
            out_v = out_p[:].rearrange("(u p) d -> p u d", p=128)

            # --- S group builder: chunk-major S[p, k*128 + c] built per
            # SG-chunk group with one DVE is_equal pass (slots vs iota).
            sgroups = {}

            def build_sgroup(gb_):
                Sw = spool.tile([128, 128 * SG], BF16, tag="S")
                sw = Sw[:]
                o = gb_ * SG
                dims = [sw.ap[0], [128, SG], [1, 128]]  # (k, c) iteration
                outap = bass.AP(sw.tensor, sw.offset, dims)
                in0 = bass.AP(sf.tensor, sf.offset + o, [sf.ap[0], [1, SG], [0, 128]])
                ii = iota8_sb[:]
                in1 = bass.AP(ii.tensor, ii.offset, [ii.ap[0], [128, SG], [1, 128]])
                nc.vector.tensor_tensor(out=outap, in0=in0, in1=in1, op=OP.is_equal)
                sgroups[gb_] = Sw
                return Sw

            nmm = (K[0] + K[1]).astype(np.int64)   # matmuls per window
            mm_done = np.zeros(WPC, np.int64)
            # PSUM tiles are bank-aligned (2KB/partition), so pack FOUR
            # windows' [128,128]f32 accumulators into one [128,512] bank
            # tile; a group of WG=8 windows uses 2 quads.
            quad_meta = {}
            quad_first = {}   # quad -> first chunk in global order; only that
                              # matmul gets start=True.  start clears the
                              # has_written bits of the WHOLE bank, so sibling
                              # windows must rely on per-element first-write
                              # overwrite semantics instead of their own start.
            for gi_, ws_ in enumerate(_groups()):
                for i_, w_ in enumerate(ws_):
                    qk = (gi_, i_ // 4)
                    quad_meta[w_] = (qk, i_ % 4)
                    if K[0, w_] + K[1, w_] > 0:
                        fc = int(cbase[0, w_] if K[0, w_] else cbase[1, w_])
                        quad_first[qk] = min(quad_first.get(qk, fc), fc)
            quads = {}

            def psum_slice(w):
                qk, slot = quad_meta[w]
                ent = quads.get(qk)
                if ent is None:
                    ent = quads[qk] = psA.tile(
                        [128, 512], F32, tag="pacc", name="pacc"
                    )
                return ent[:, slot * 128 : (slot + 1) * 128]

            def finalize(w):
                wsl = slice(w * 128, (w + 1) * 128)
                att = fpool.tile([128, 128], BF16, tag="att")
                if int(nmm[w]):
                    nc.vector.tensor_tensor(
                        out=att[:], in0=psum_slice(w), in1=xst_sb[:, wsl],
                        op=OP.add,
                    )
                else:
                    nc.vector.tensor_copy(att[:], xst_sb[:, wsl])
                po = psO.tile([128, 128], F32, tag="po")
                nc.tensor.matmul(
                    po[:], lhsT=att[:], rhs=wt_sb[:], start=True, stop=True
                )
                ot = fpool.tile([128, 128], BF16, tag="ot")
                nc.scalar.activation(
                    ot[:], po[:], AF.Relu, scale=dis_s[:, w : w + 1]
                )
                nc.sync.dma_start(out_v[:, w, :], ot[:])

            qrr = 0
            for gi, ws in enumerate(groups):
                for p in range(2):
                    r0, rn = runs[(gi, p)]
                    done = 0
                    while done < rn:
                        nch = min(GB, rn - done)
                        cc = r0 + done
                        ptile, pbase = gidx_sb[part_of_g[gi]]
                        lofs = (cc - pbase) * 8
                        gt = gpool.tile([128, GB * 128], BF16, tag="gt")
                        gv = gt[:, : nch * 128].rearrange(
                            "p (b e) -> p b e", e=128
                        )
                        nc.gpsimd.dma_gather(
                            gv,
                            tables[p],
                            ptile[:, lofs : lofs + nch * 8],
                            nch * 128,
                            nch * 128,
                            D,
                            queue_num=qrr % NQ,
                        )
                        qrr += 1
                        for k in range(nch):
                            gidx_c = cc + k
                            w = int(owner[gidx_c])
                            gb_, kk = divmod(gidx_c, SG)
                            Sw = sgroups.get(gb_)
                            if Sw is None:
                                Sw = build_sgroup(gb_)
                            nc.tensor.matmul(
                                psum_slice(w),
                                lhsT=gt[:, k * 128 : (k + 1) * 128],
                                rhs=Sw[:, kk * 128 : (kk + 1) * 128],
                                start=bool(gidx_c == quad_first[quad_meta[w][0]]),
                                stop=bool(mm_done[w] == nmm[w] - 1),
                                skip_group_check=True,
                            )
                            mm_done[w] += 1
                            if mm_done[w] == nmm[w]:
                                finalize(w)
                        done += nch
                # windows with zero chunks (nmm==0) still need an output
                for w in ws:
                    if nmm[w] == 0 and mm_done[w] == 0:
                        mm_done[w] = -1
                        finalize(w)

    nc.compile()
    return nc


# ---------------------------------------------------------------- runner
_CACHE = {}


def _get_program(K):
    key = K.tobytes()
    if key not in _CACHE:
        _CACHE[key] = build_program(K)
    return _CACHE[key]


def make_in_maps(x, W, prep):
    x = np.asarray(x, np.float32)
    # gather table = h = x * dis (host-prescaled, O(N) prep)
    h = x * prep["dis"][:, None].astype(np.float32)
    xb = np.zeros((NROWS, D), ml_dtypes.bfloat16)
    xb[:N] = h.astype(ml_dtypes.bfloat16)
    Wt = np.ascontiguousarray(np.asarray(W, np.float32).T).astype(
        ml_dtypes.bfloat16
    )
    # iota8[p, k*128 + c] = c  (chunk-major)
    iota8 = np.tile(
        np.tile(np.arange(128, dtype=np.float32), SG)[None, :], (128, 1)
    ).astype(ml_dtypes.bfloat16)
    in_maps = []
    for c in range(C):
        cd = prep["cores"][c]
        xst = np.zeros((D, NPAD), ml_dtypes.bfloat16)
        xst[:, :NPC] = x[c * NPC : (c + 1) * NPC].T.astype(ml_dtypes.bfloat16)
        # permute window blocks of columns to the processing order
        xst = (
            xst.reshape(D, WPC, 128)[:, cd["perm"], :].reshape(D, NPAD).copy()
        )
        in_maps.append(
            {
                "xb": xb,
                "xst": xst,
                "wt": Wt,
                "iota8": iota8,
                "rp0s": cd["rp0s"],
                "rp1s": cd["rp1s"],
                "gidx": cd["gidx"],
                "slots": cd["slots"],
            }
        )
    return in_maps


def run_spmd(x, edge_index, W, trace=False, **spmd_kwargs):
    prep = host_prep(edge_index)
    nc = _get_program(prep["K"])
    in_maps = make_in_maps(x, W, prep)
    res = run_bass_kernel_spmd(nc, in_maps, list(range(C)), trace=trace, **spmd_kwargs)
    parts = []
    for c in range(C):
        ob = np.asarray(res.results[c]["out"], np.float32).reshape(WPC, 128, D)
        inv = np.empty(WPC, np.int64)
        inv[prep["cores"][c]["perm"]] = np.arange(WPC)
        parts.append(ob[inv].reshape(NPAD, D)[:NPC])
    return np.concatenate(parts, axis=0), res


def kernel(x, edge_index, N=None, W=None, **_):
    out, _res = run_spmd(np.asarray(x), np.asarray(edge_index), np.asarray(W))
    return out


# revision 42
# speedup vs baseline: 1.0583x; 1.0583x over previous
"""GCN layer (message passing + linear + ReLU) on 8 Trainium2 NeuronCores.

out = relu(((scatter_add(h[src] -> dst) + x) * dis) @ W.T),
h = x * dis,  dis = rsqrt(deg + 1),  deg = in-degree via dst counts.

Strategy (SPMD, one program on 8 cores):
  - Nodes sharded contiguously: core c owns rows [c*6250, (c+1)*6250).
  - Host partitions edges by dst owner and sorts by dst (index-only work);
    degree reaches the device as CSR rowptr slices, so deg = rowptr diff
    and dis = 1/sqrt(deg+1) are computed on device in f32.
  - The gather table is h = x*dis in bf16 (host-prescaled, O(N) prep like
    the reference's h): each edge's h[src] is one 256B dma_gather row, so
    the scatter matrix S is a pure one-hot (slot==iota) built with a
    single DVE is_equal per group -- no per-edge scale multiply.
  - Scatter-add on-chip: edges sorted by dst fall into windows of 128
    owned nodes; per 128-edge chunk S[e, slot] = (slot==dst-base) is
    built on DVE and the PE accumulates gt.T @ S = agg.T [feat, slot]
    into the window's PSUM tile.
  - Windows are processed in GROUPS of WG: within a (group, pass) the
    chunk runs of all WG windows are packed back-to-back so gather calls
    (<=1024 idx each) span window boundaries.  This cuts the number of
    dma_gather instructions from ~131 to ~85: every Q7 core pair walks
    the whole GpSimd instruction stream (non-owning pairs pay an
    idle-skip per instruction), so fewer+fuller calls raise the pairs'
    useful descriptor-generation throughput, which paces the kernel.
  - Edges inside each (pass, window) block are sorted by src so the
    random 256B HBM reads of a call walk ascending addresses (better
    row-buffer/bank locality on the drain side).
  - int16 gather indices can't span 50k rows, so each window's edges are
    processed against table halves h[:32768] / h[32768:] (pass lo/hi).
  - Gather calls round-robin over 4 SWDGE queues (queue q is served by Q7
    cores 2q/2q+1 -- tx/rx descriptor streams in parallel).
  - Pad gather slots use idx 0 (a real row, masked by S=0).  -1 pad +
    ucode trailing-trim was tried and WEDGES THE DEVICE: the NX decode
    reserves ring space from num_idxs_reg (untrimmed) while the Q7
    writes fewer descriptors, desyncing the SWDGE ring tail so SDMA
    executes stale descriptor slots.  Trimming needs per-core count
    registers, which stall the Pool decode pipeline (baseline note).
  - gidx/xst uploads are split so the first gather/finalize only waits
    for a small first segment.
  - Measured (NTFF traces): the GpSimd extended-instruction stream is the
    pacer -- the Q7 cluster retires gather instructions near-serially at
    ~2.15ns/idx + ~0.3us/call (dispatches run 6+ calls ahead; SDMA queues
    are ~50% idle).  Tried and NOT better (likely noise-dominated, device
    shows +-10-20us run variance): single_packet=False, 48KB SWDGE
    scratch rings, tapered trailing groups, group-batched output DMAs,
    warmup gathers (actively harmful: they serialize on the cluster).
  - Finalize per window, fused right after its last matmul: att =
    (psum + xT) in bf16, po = att.T @ W.T via PE (no transpose needed:
    agg is feature-major), out = relu(po * dis_dst) via ACT per-partition
    scale, then DMA out.
Chunk counts per (pass, window) are maxed over cores so the single SPMD
program fits every core; shorter cores pad with slot=255 / idx=0 chunks.
"""
import numpy as np
import ml_dtypes

from concourse import bacc, bass, mybir, tile
from concourse.bass_utils import run_bass_kernel_spmd

F32 = mybir.dt.float32
BF16 = mybir.dt.bfloat16
I32 = mybir.dt.int32
I16 = mybir.dt.int16
AF = mybir.ActivationFunctionType
OP = mybir.AluOpType

N = 50000
E = 600000
D = 128
C = 8                      # cores
NPC = N // C               # 6250 nodes per core
WPC = (NPC + 127) // 128   # 49 windows per core
NPAD = WPC * 128           # 6272 padded shard rows
NT_G = (N + 127) // 128    # 391 global node tiles
NROWS = NT_G * 128         # 50048 padded table rows
SPLIT = 32768              # src table split for int16 gather indices
PASS_BOUNDS = [(0, SPLIT), (SPLIT, N)]
GB = 8                     # max chunks per dma_gather call (1024 idxs; >1024
                           # descriptors per SWDGE call crashes the device)
SG = 16                    # chunks per S-group build
NQ = 4                     # SWDGE queues used round-robin
WG = 8                     # windows per processing group (gather-call packing
                           # unit; quad-packed PSUM accumulators)
XST_PARTS = 7              # xst upload split (7 windows each)


def _groups():
    return [list(range(g0, min(g0 + WG, WPC))) for g0 in range(0, WPC, WG)]


def _chunk_layout(K):
    """Chunk layout in (group, pass, window) order.

    Returns cbase[p, w] = global chunk index of window w's pass-p block,
    runs[(g, p)] = (start_chunk, n_chunks) of the packed per-group run,
    and TC = total chunks."""
    K = np.asarray(K)
    cbase = np.zeros((2, WPC), np.int64)
    runs = {}
    cb = 0
    for g, ws in enumerate(_groups()):
        for p in range(2):
            r0 = cb
            for w in ws:
                cbase[p, w] = cb
                cb = int(cb + K[p, w])
            runs[(g, p)] = (int(r0), int(cb - r0))
    return cbase, runs, int(cb)


# ---------------------------------------------------------------- host prep
def host_prep(edge_index):
    src = np.asarray(edge_index[0], dtype=np.int64)
    dst = np.asarray(edge_index[1], dtype=np.int64)
    order = np.argsort(dst, kind="stable")
    ss_all = src[order]
    dd_all = dst[order]
    counts = np.bincount(dst, minlength=N)
    rowptr = np.zeros(N + 1, np.int64)
    rowptr[1:] = np.cumsum(counts)
    dis = 1.0 / np.sqrt(counts.astype(np.float64) + 1.0)  # rsqrt(deg+1)

    per_core = []
    need = np.zeros((C, 2, WPC), np.int64)
    for c in range(C):
        e0, e1 = rowptr[c * NPC], rowptr[(c + 1) * NPC]
        ss, dd = ss_all[e0:e1], dd_all[e0:e1]
        per_core.append((ss, dd))
        for p, (lo, hi) in enumerate(PASS_BOUNDS):
            m = (ss >= lo) & (ss < hi)
            w = (dd[m] - c * NPC) // 128
            need[c, p] = np.bincount(w, minlength=WPC)
    # Each core processes ITS windows sorted by edge count (descending).
    # Aligning the order statistics across cores shrinks the max-over-core
    # chunk counts K, and the final processed window is everyone's
    # smallest (shorter tail after the last gather).
    perm = np.argsort(-need.sum(axis=1), axis=1, kind="stable")  # [C, WPC]
    need_s = np.take_along_axis(need, perm[:, None, :], axis=2)
    K = np.ceil(need_s.max(axis=0) / 128).astype(np.int64)  # [2, WPC]

    cbase, runs, TC = _chunk_layout(K)
    TC8 = ((TC + SG - 1) // SG) * SG

    cores = []
    for c in range(C):
        ss, dd = per_core[c]
        slot_of_w = np.empty(WPC, np.int64)
        slot_of_w[perm[c]] = np.arange(WPC)
        g = np.zeros(TC * 128, np.int64)
        s = np.full(TC8 * 128, 255, np.int64)
        for p, (lo, hi) in enumerate(PASS_BOUNDS):
            m = (ss >= lo) & (ss < hi)
            sg = ss[m]                       # global src id
            dloc = dd[m] - c * NPC
            wslot = slot_of_w[dloc // 128]
            # per-window blocks, src-ascending inside each block
            o2 = np.lexsort((sg, wslot))
            sg, dloc, wslot = sg[o2], dloc[o2], wslot[o2]
            cnt = np.bincount(wslot, minlength=WPC)
            ofs = np.zeros(WPC, np.int64)
            np.cumsum(cnt[:-1], out=ofs[1:])
            pos = cbase[p, wslot] * 128 + (np.arange(len(sg)) - ofs[wslot])
            g[pos] = sg - lo                 # index into table half
            s[pos] = dloc - (dloc // 128) * 128
        d = {}
        # gather idx layout [128, TC*8]: stream pos j at [j%16, j//16],
        # replicated across the 8 groups of 16 partitions (each SWDGE queue's
        # tx/rx Q7 core pair reads its own partition group).
        d["gidx"] = np.tile(g.reshape(-1, 16).T.astype(np.int16), (8, 1)).copy()
        # slot layout [128, TC8]: stream pos j at [j%128, j//128]; values
        # <= 255 are exact in bf16 so the S-build is a single is_equal.
        d["slots"] = s.reshape(-1, 128).T.astype(ml_dtypes.bfloat16).copy()
        n0 = c * NPC
        rpv = np.full(NPAD + 1, rowptr[min((c + 1) * NPC, N)], np.int64)
        rpv[: NPC + 1] = rowptr[n0 : n0 + NPC + 1]
        rp0m = rpv[:NPAD].reshape(WPC, 128)[perm[c]]
        rp1m = rpv[1 : NPAD + 1].reshape(WPC, 128)[perm[c]]
        d["rp0s"] = rp0m.T.astype(np.int32).copy()
        d["rp1s"] = rp1m.T.astype(np.int32).copy()
        d["perm"] = perm[c]
        cores.append(d)
    return dict(K=K, TC=TC, TC8=TC8, cores=cores, dis=dis)


# ---------------------------------------------------------------- program
def build_program(K):
    K = np.asarray(K)
    cbase, runs, TC = _chunk_layout(K)
    TC8 = ((TC + SG - 1) // SG) * SG
    groups = _groups()

    # owner window of every global chunk (program-fixed)
    owner = np.zeros(TC, np.int64)
    for w in range(WPC):
        for p in range(2):
            owner[cbase[p, w] : cbase[p, w] + K[p, w]] = w

    # gidx upload parts over group ranges
    ngrp = len(groups)
    splits = [s for s in [0, 1, 2, 4, ngrp] if s <= ngrp]
    if splits[-1] != ngrp:
        splits.append(ngrp)
    gparts = []
    for i in range(len(splits) - 1):
        ga, gb_ = splits[i], splits[i + 1]
        c0 = runs[(ga, 0)][0]
        c1 = TC if gb_ >= ngrp else runs[(gb_, 0)][0]
        gparts.append((ga, gb_, c0, c1))

    nc = bacc.Bacc(
        None, target_bir_lowering=False, debug=False, num_swdge_queues=NQ
    )

    x_p = nc.dram_tensor("xb", [NROWS, D], BF16, kind="ExternalInput")
    xst_p = nc.dram_tensor("xst", [D, NPAD], BF16, kind="ExternalInput")
    wt_p = nc.dram_tensor("wt", [D, D], BF16, kind="ExternalInput")
    iota8_p = nc.dram_tensor("iota8", [128, 128 * SG], BF16, kind="ExternalInput")
    rp0s_p = nc.dram_tensor("rp0s", [128, WPC], I32, kind="ExternalInput")
    rp1s_p = nc.dram_tensor("rp1s", [128, WPC], I32, kind="ExternalInput")
    gidx_p = nc.dram_tensor("gidx", [128, TC * 8], I16, kind="ExternalInput")
    slots_p = nc.dram_tensor("slots", [128, TC8], BF16, kind="ExternalInput")
    out_p = nc.dram_tensor("out", [NPAD, D], BF16, kind="ExternalOutput")

    with tile.TileContext(nc) as tc:
        with (
            tc.tile_pool(name="const", bufs=1) as cpool,
            tc.tile_pool(name="gather", bufs=20) as gpool,
            tc.tile_pool(name="sel", bufs=6) as spool,
            tc.tile_pool(name="fin", bufs=3) as fpool,
            tc.tile_pool(name="psA", bufs=4, space="PSUM") as psA,
            tc.tile_pool(name="psO", bufs=2, space="PSUM") as psO,
        ):
            # --- uploads; gidx parts on the sync queue (first part first so
            # gathers start early), metadata then xst on the scalar queue.
            # slots + iota go first on the scalar queue: they gate every
            # S-group build.
            gidx_sb = {}
            part_of_g = {}
            for i, (ga, gb_, c0, c1) in enumerate(gparts):
                gt_ = cpool.tile([128, (c1 - c0) * 8], I16, tag=f"gidx{i}")
                gidx_sb[i] = (gt_, c0)
                for gg in range(ga, gb_):
                    part_of_g[gg] = i
            nc.sync.dma_start(
                gidx_sb[0][0][:], gidx_p[:, gparts[0][2] * 8 : gparts[0][3] * 8]
            )
            sf = cpool.tile([128, TC8], BF16, tag="sf")
            nc.scalar.dma_start(sf[:], slots_p[:])
            iota8_sb = cpool.tile([128, 128 * SG], BF16, tag="iota8")
            nc.scalar.dma_start(iota8_sb[:], iota8_p[:])
            wt_sb = cpool.tile([128, 128], BF16, tag="wt")
            nc.scalar.dma_start(wt_sb[:], wt_p[:])
            r0i = cpool.tile([128, WPC], I32, tag="r0i")
            nc.scalar.dma_start(r0i[:], rp0s_p[:])
            r1i = cpool.tile([128, WPC], I32, tag="r1i")
            nc.scalar.dma_start(r1i[:], rp1s_p[:])

            # dis_dst = 1/sqrt(deg+1) from rowptr diffs, [128, WPC] f32
            r0f = cpool.tile([128, WPC], F32, tag="r0f")
            nc.vector.tensor_copy(r0f[:], r0i[:])
            r1f = cpool.tile([128, WPC], F32, tag="r1f")
            nc.vector.tensor_copy(r1f[:], r1i[:])
            dg = cpool.tile([128, WPC], F32, tag="dg")
            nc.vector.tensor_tensor(out=dg[:], in0=r1f[:], in1=r0f[:], op=OP.subtract)
            nc.vector.tensor_scalar_add(out=dg[:], in0=dg[:], scalar1=1.0)
            rc = cpool.tile([128, WPC], F32, tag="rc")
            nc.vector.reciprocal(rc[:], dg[:])
            dis_s = cpool.tile([128, WPC], F32, tag="dis")
            nc.scalar.activation(dis_s[:], rc[:], AF.Sqrt)

            # remaining gidx parts (sync queue), xst parts (scalar queue)
            for i in range(1, len(gparts)):
                t, c0 = gidx_sb[i]
                nc.sync.dma_start(t[:], gidx_p[:, c0 * 8 : gparts[i][3] * 8])
            xst_sb = cpool.tile([128, NPAD], BF16, tag="xst")
            xw = (WPC + XST_PARTS - 1) // XST_PARTS  # windows per xst part
            for i in range(XST_PARTS):
                a, b = i * xw * 128, min((i + 1) * xw * 128, NPAD)
                nc.scalar.dma_start(xst_sb[:, a:b], xst_p[:, a:b])

            tables = [x_p[0:SPLIT, :], x_p[SPLIT:NROWS, :]]
            out_v = out_p[:].rearrange("(u p) d -> p u d", p=128)

            # --- S group builder: chunk-major S[p, k*128 + c] built per
            # SG-chunk group with one DVE is_equal pass (slots vs iota).
            sgroups = {}

            def build_sgroup(gb_):
                Sw = spool.tile([128, 128 * SG], BF16, tag="S")
                sw = Sw[:]
                o = gb_ * SG
                dims = [sw.ap[0], [128, SG], [1, 128]]  # (k, c) iteration
                outap = bass.AP(sw.tensor, sw.offset, dims)
                in0 = bass.AP(sf.tensor, sf.offset + o, [sf.ap[0], [1, SG], [0, 128]])
                ii = iota8_sb[:]
                in1 = bass.AP(ii.tensor, ii.offset, [ii.ap[0], [128, SG], [1, 128]])
                nc.vector.tensor_tensor(out=outap, in0=in0, in1=in1, op=OP.is_equal)
                sgroups[gb_] = Sw
                return Sw

            nmm = (K[0] + K[1]).astype(np.int64)   # matmuls per window
            mm_done = np.zeros(WPC, np.int64)
            # PSUM tiles are bank-aligned (2KB/partition), so pack FOUR
            # windows' [128,128]f32 accumulators into one [128,512] bank
            # tile; a group of WG=8 windows uses 2 quads.
            quad_meta = {}
            quad_first = {}   # quad -> first chunk in global order; only that
                              # matmul gets start=True.  start clears the
                              # has_written bits of the WHOLE bank, so sibling
                              # windows must rely on per-element first-write
                              # overwrite semantics instead of their own start.
            for gi_, ws_ in enumerate(_groups()):
                for i_, w_ in enumerate(ws_):
                    qk = (gi_, i_ // 4)
                    quad_meta[w_] = (qk, i_ % 4)
                    if K[0, w_] + K[1, w_] > 0:
                        fc = int(cbase[0, w_] if K[0, w_] else cbase[1, w_])
                        quad_first[qk] = min(quad_first.get(qk, fc), fc)
            quads = {}

            def psum_slice(w):
                qk, slot = quad_meta[w]
                ent = quads.get(qk)
                if ent is None:
                    ent = quads[qk] = psA.tile(
                        [128, 512], F32, tag="pacc", name="pacc"
                    )
                return ent[:, slot * 128 : (slot + 1) * 128]

            def finalize(w):
                wsl = slice(w * 128, (w + 1) * 128)
                att = fpool.tile([128, 128], BF16, tag="att")
                if int(nmm[w]):
                    nc.vector.tensor_tensor(
                        out=att[:], in0=psum_slice(w), in1=xst_sb[:, wsl],
                        op=OP.add,
                    )
                else:
                    nc.vector.tensor_copy(att[:], xst_sb[:, wsl])
                po = psO.tile([128, 128], F32, tag="po")
                nc.tensor.matmul(
                    po[:], lhsT=att[:], rhs=wt_sb[:], start=True, stop=True
                )
                ot = fpool.tile([128, 128], BF16, tag="ot")
                nc.scalar.activation(
                    ot[:], po[:], AF.Relu, scale=dis_s[:, w : w + 1]
                )
                nc.sync.dma_start(out_v[:, w, :], ot[:])

            qrr = 0
            for gi, ws in enumerate(groups):
                for p in range(2):
                    r0, rn = runs[(gi, p)]
                    done = 0
                    while done < rn:
                        nch = min(GB, rn - done)
                        cc = r0 + done
                        ptile, pbase = gidx_sb[part_of_g[gi]]
                        lofs = (cc - pbase) * 8
                        gt = gpool.tile([128, GB * 128], BF16, tag="gt")
                        gv = gt[:, : nch * 128].rearrange(
                            "p (b e) -> p b e", e=128
                        )
                        nc.gpsimd.dma_gather(
                            gv,
                            tables[p],
                            ptile[:, lofs : lofs + nch * 8],
                            nch * 128,
                            nch * 128,
                            D,
                            queue_num=qrr % NQ,
                        )
                        qrr += 1
                        for k in range(nch):
                            gidx_c = cc + k
                            w = int(owner[gidx_c])
                            gb_, kk = divmod(gidx_c, SG)
                            Sw = sgroups.get(gb_)
                            if Sw is None:
                                Sw = build_sgroup(gb_)
                            nc.tensor.matmul(
                                psum_slice(w),
                                lhsT=gt[:, k * 128 : (k + 1) * 128],
                                rhs=Sw[:, kk * 128 : (kk + 1) * 128],
                                start=bool(gidx_c == quad_first[quad_meta[w][0]]),
                                stop=bool(mm_done[w] == nmm[w] - 1),
                                skip_group_check=True,
                            )
                            mm_done[w] += 1
                            if mm_done[w] == nmm[w]:
                                finalize(w)
                        done += nch
                # windows with zero chunks (nmm==0) still need an output
                for w in ws:
                    if nmm[w] == 0 and mm_done[w] == 0:
                        mm_done[w] = -1
                        finalize(w)

    nc.compile()
    return nc


# ---------------------------------------------------------------- runner
_CACHE = {}


def _get_program(K):
    key = K.tobytes()
    if key not in _CACHE:
        _CACHE[key] = build_program(K)
    return _CACHE[key]


def make_in_maps(x, W, prep):
    x = np.asarray(x, np.float32)
    # gather table = h = x * dis (host-prescaled, O(N) prep)
    h = x * prep["dis"][:, None].astype(np.float32)
    xb = np.zeros((NROWS, D), ml_dtypes.bfloat16)
    xb[:N] = h.astype(ml_dtypes.bfloat16)
    Wt = np.ascontiguousarray(np.asarray(W, np.float32).T).astype(
        ml_dtypes.bfloat16
    )
    # iota8[p, k*128 + c] = c  (chunk-major)
    iota8 = np.tile(
        np.tile(np.arange(128, dtype=np.float32), SG)[None, :], (128, 1)
    ).astype(ml_dtypes.bfloat16)
    in_maps = []
    for c in range(C):
        cd = prep["cores"][c]
        xst = np.zeros((D, NPAD), ml_dtypes.bfloat16)
        xst[:, :NPC] = x[c * NPC : (c + 1) * NPC].T.astype(ml_dtypes.bfloat16)
        # permute window blocks of columns to the processing order
        xst = (
            xst.reshape(D, WPC, 128)[:, cd["perm"], :].reshape(D, NPAD).copy()
        )
        in_maps.append(
            {
                "xb": xb,
                "xst": xst,
                "wt": Wt,
                "iota8": iota8,
                "rp0s": cd["rp0s"],
                "rp1s": cd["rp1s"],
                "gidx": cd["gidx"],
                "slots": cd["slots"],
            }
        )
    return in_maps


def run_spmd(x, edge_index, W, trace=False, **spmd_kwargs):
    prep = host_prep(edge_index)
    nc = _get_program(prep["K"])
    in_maps = make_in_maps(x, W, prep)
    res = run_bass_kernel_spmd(nc, in_maps, list(range(C)), trace=trace, **spmd_kwargs)
    parts = []
    for c in range(C):
        ob = np.asarray(res.results[c]["out"], np.float32).reshape(WPC, 128, D)
        inv = np.empty(WPC, np.int64)
        inv[prep["cores"][c]["perm"]] = np.arange(WPC)
        parts.append(ob[inv].reshape(NPAD, D)[:NPC])
    return np.concatenate(parts, axis=0), res


def kernel(x, edge_index, N=None, W=None, **_):
    out, _res = run_spmd(np.asarray(x), np.asarray(edge_index), np.asarray(W))
    return out
